# revision 7
# baseline (speedup 1.0000x reference)
"""Bass/Trainium2 kernel for nn_MultiHeadAttention (B=2,S=2048,D=1024,H=16).

Sharding: 8 cores = 2 batches x 4 head-groups (Megatron-style). Each core
computes 4 heads of one batch: QKV projections from X^T (replicated per
batch), causal softmax (alpha written to HBM), context, and a partial
output projection. Host sums the 4 partial outputs per batch (the W_o
all-reduce) and concatenates alpha shards. The causal upper triangle of
alpha is never written on device: output buffers are pre-zeroed.

All matmuls run in float32r (full-rate on the PE at N>=256, ~1e-4 rel
error); accumulation and softmax are fp32.
"""
import numpy as np

import concourse.bass as bass
import concourse.mybir as mybir
from concourse import bacc
from concourse.tile import TileContext
from concourse.bass_utils import run_bass_kernel_spmd

F32 = mybir.dt.float32
R32 = mybir.dt.float32r
AFT = mybir.ActivationFunctionType

B, S, D, H = 2, 2048, 1024, 16
DK = D // H          # 64
HPC = 4              # heads per core
NT = S // 128        # 16 q/k tiles
NEGV = -3.0e37

_BUILD_CACHE = {}


def _build(causal: bool, padded: bool, timing: bool = False, nreps: int = 1):
    nc = bacc.Bacc()
    XT_ = nc.declare_dram_parameter("XT", [D, S], R32, isOutput=False)
    WQT_ = nc.declare_dram_parameter("WQT", [D, 256], R32, isOutput=False)
    WKT_ = nc.declare_dram_parameter("WKT", [D, 256], R32, isOutput=False)
    WVT_ = nc.declare_dram_parameter("WVT", [D, 256], R32, isOutput=False)
    WOT_ = nc.declare_dram_parameter("WOT", [256, D], R32, isOutput=False)
    BIA_ = nc.declare_dram_parameter("BIA", [1, 768], R32, isOutput=False)
    PADB_ = nc.declare_dram_parameter("PADB", [128, NT], F32, isOutput=False)
    PADR_ = nc.declare_dram_parameter("PADR", [1, S], R32, isOutput=False)
    NEG_ = nc.declare_dram_parameter("NEG", [128, 128], R32, isOutput=False)
    NEGT_ = nc.declare_dram_parameter("NEGT", [128, 128], R32, isOutput=False)
    IDNR_ = nc.declare_dram_parameter("IDNR", [128, 128], R32, isOutput=False)
    IDN32_ = nc.declare_dram_parameter("IDN32", [128, 128], F32, isOutput=False)
    if timing:
        ALP_ = nc.dram_tensor("alp_i", [HPC, S, S], F32)
        OUT_ = nc.dram_tensor("out_i", [S, D], F32)
        PRB_ = nc.declare_dram_parameter("PRB", [128, 16], F32, isOutput=True)
    else:
        ALP_ = nc.declare_dram_parameter("ALP", [HPC, S, S], F32, isOutput=True)
        OUT_ = nc.declare_dram_parameter("OUT", [S, D], F32, isOutput=True)
        PRB_ = None

    with TileContext(nc) as tc:
        import contextlib
        est = contextlib.ExitStack()
        pw = est.enter_context(tc.tile_pool(name="pw", bufs=1))
        pp = est.enter_context(tc.tile_pool(name="pp", bufs=1, space="PSUM"))

        # ---- persistent tiles (live through attention) -------------------
        wo = pw.tile([64, 4 * 1024], R32, name="wo")
        padb = pw.tile([128, NT], F32, name="padb")
        neg = pw.tile([128, 128], R32, name="neg")
        negt = pw.tile([128, 128], R32, name="negt")
        idnr = pw.tile([128, 128], R32, name="idnr")
        idn32 = pw.tile([128, 128], F32, name="idn32")
        ones32 = pw.tile([1, 512], F32, name="ones32")
        onesr = pw.tile([1, 512], R32, name="onesr")
        padr = pw.tile([1, S], R32, name="padr") if padded else None
        qt = [pw.tile([128, S], R32, name=f"qt{p}") for p in range(2)]
        kt = [pw.tile([128, S], R32, name=f"kt{p}") for p in range(2)]
        vsb = pw.tile([128, NT * 256], R32, name="vsb")

        def rearr8(ap):
            return ap.rearrange("(t p) c -> p t c", p=128)

        def load_consts():
            nc.sync.dma_start(out=wo[:].rearrange("p (t c) -> p t c", t=4),
                              in_=WOT_[:].rearrange("(t p) c -> p t c", p=64))
            nc.sync.dma_start(out=padb[:], in_=PADB_[:])
            if padded:
                nc.sync.dma_start(out=padr[:], in_=PADR_[:])
            nc.sync.dma_start(out=neg[:], in_=NEG_[:])
            nc.sync.dma_start(out=negt[:], in_=NEGT_[:])
            nc.sync.dma_start(out=idnr[:], in_=IDNR_[:])
            nc.sync.dma_start(out=idn32[:], in_=IDN32_[:])
            nc.vector.memset(ones32[:], 1.0)
            nc.vector.tensor_copy(out=onesr[:], in_=ones32[:])

        def qkv_phase(px):
            wq = px.tile([128, 8 * 256], R32, name="wq")
            wk = px.tile([128, 8 * 256], R32, name="wk")
            wv = px.tile([128, 8 * 256], R32, name="wv")
            bia = px.tile([1, 768], R32, name="bia")
            xt = px.tile([128, 8 * S], R32, name="xt")
            nc.sync.dma_start(out=wq[:].rearrange("p (t c) -> p t c", t=8),
                              in_=rearr8(WQT_[:]))
            nc.sync.dma_start(out=wk[:].rearrange("p (t c) -> p t c", t=8),
                              in_=rearr8(WKT_[:]))
            nc.sync.dma_start(out=wv[:].rearrange("p (t c) -> p t c", t=8),
                              in_=rearr8(WVT_[:]))
            nc.sync.dma_start(out=bia[:], in_=BIA_[:])
            nc.sync.dma_start(out=xt[:].rearrange("p (t c) -> p t c", t=8),
                              in_=rearr8(XT_[:]))

            def xtt(t):
                return xt[:, t * S:(t + 1) * S]

            # QT/KT: stationary = weight block, moving = X^T s-chunk
            for (w, dst, bofs) in ((wq, qt, 0), (wk, kt, 256)):
                for p in range(2):
                    for sc in range(4):
                        ps_ = pp.tile([128, 512], F32, name="mmq", tag="mm",
                                      bufs=2)
                        for t in range(8):
                            nc.tensor.matmul(
                                ps_[:],
                                w[:, t * 256 + p * 128:t * 256 + p * 128 + 128],
                                xtt(t)[:, sc * 512:sc * 512 + 512],
                                start=(t == 0), stop=False)
                        nc.tensor.matmul(
                            ps_[:],
                            bia[0:1, bofs + p * 128:bofs + p * 128 + 128],
                            onesr[:], start=False, stop=True)
                        nc.vector.tensor_copy(
                            out=dst[p][:, sc * 512:sc * 512 + 512], in_=ps_[:])
            # V natural [s, dk_local]
            for st in range(NT):
                ps_ = pp.tile([128, 512], F32, name="mmv", tag="mm", bufs=2)
                for t in range(8):
                    nc.tensor.matmul(
                        ps_[:, 0:256],
                        xtt(t)[:, st * 128:st * 128 + 128],
                        wv[:, t * 256:t * 256 + 256],
                        start=(t == 0), stop=False)
                nc.tensor.matmul(ps_[:, 0:256], onesr[0:1, 0:128],
                                 bia[0:1, 512:768], start=False, stop=True)
                nc.vector.tensor_copy(out=vsb[:, st * 256:st * 256 + 256],
                                      in_=ps_[:, 0:256])

        def nat_path(pa, p, hl, rsum, sc2):
            """Natural-layout scores -> exp(+rowsum) -> normalize -> DMA."""
            ro = (hl % 2) * 64
            for qi in range(NT):
                wf = (qi + 1) * 128 if causal else S
                ntile = (wf + 1023) // 1024
                rowt = []
                for ti in range(ntile):
                    w0 = ti * 1024
                    w1 = min(wf, w0 + 1024)
                    ps_ = pp.tile([128, 1024], F32, name="sa", tag="sa",
                                  bufs=2)
                    rowt.append((ps_, w0, w1))
                    for c0 in range(w0, w1, 512):
                        c1 = min(w1, c0 + 512)
                        has_neg = causal and (c1 == wf)
                        nc.tensor.matmul(
                            ps_[:, c0 - w0:c1 - w0],
                            qt[p][ro:ro + 64, qi * 128:qi * 128 + 128],
                            kt[p][ro:ro + 64, c0:c1],
                            start=True, stop=not padded and not has_neg)
                        if padded:
                            nc.tensor.matmul(
                                ps_[:, c0 - w0:c1 - w0],
                                onesr[0:1, 0:128], padr[0:1, c0:c1],
                                start=False, stop=not has_neg)
                        if has_neg:
                            nc.tensor.matmul(
                                ps_[:, wf - 128 - w0:wf - w0],
                                idnr[:], neg[:], start=False, stop=True)
                # exp with free row-sum accumulation
                alb = pa.tile([128, S], F32, name="alb", tag="alb", bufs=2)
                col = hl * 16 + qi
                for ti, (ps_, w0, w1) in enumerate(rowt):
                    acc = (rsum[:, col:col + 1] if ntile == 1
                           else sc2[:, ti:ti + 1])
                    nc.scalar.activation(
                        alb[:, w0:w1], ps_[:, 0:w1 - w0], AFT.Exp,
                        scale=0.125, accum_out=acc)
                if ntile > 1:
                    nc.vector.tensor_tensor(
                        out=rsum[:, col:col + 1], in0=sc2[:, 0:1],
                        in1=sc2[:, 1:2], op=mybir.AluOpType.add)
                rc = pa.tile([128, 1], F32, name="rc", tag="rc", bufs=3)
                nc.vector.reciprocal(out=rc[:], in_=rsum[:, col:col + 1])
                eng = nc.gpsimd if (qi % 2 == 0) else nc.vector
                eng.tensor_scalar_mul(alb[:, 0:wf], alb[:, 0:wf], rc[:, 0:1])
                nc.sync.dma_start(
                    out=ALP_[hl, qi * 128:qi * 128 + 128, 0:wf],
                    in_=alb[:, 0:wf])

        def bcast_recip(pa, hl, rsum):
            """[64, S] broadcast tile of 1/rowsum for one head."""
            bcast = pa.tile([64, S], F32, name="bcast", tag="bcast", bufs=2)
            pt_ = pp.tile([128, 512], F32, name="ptr", tag="mm", bufs=2)
            nc.tensor.transpose(pt_[0:16, 0:128],
                                rsum[:, hl * 16:hl * 16 + 16], idn32[:])
            rrt = pa.tile([16, 128], R32, name="rrt", tag="rrt", bufs=2)
            with nc.allow_low_precision(reason="f32r recip feeds matmul"):
                nc.vector.reciprocal(out=rrt[:], in_=pt_[0:16, 0:128])
            row = pa.tile([1, S], R32, name="row", tag="row", bufs=2)
            nc.sync.dma_start(out=row[:], in_=rrt[:])
            for sc in range(4):
                pb_ = pp.tile([128, 512], F32, name="pb", tag="mm", bufs=2)
                nc.tensor.matmul(pb_[0:64, :], onesr[0:1, 0:64],
                                 row[0:1, sc * 512:sc * 512 + 512],
                                 start=True, stop=True)
                nc.vector.tensor_copy(
                    out=bcast[:, sc * 512:sc * 512 + 512], in_=pb_[0:64, :])
            return bcast

        def t_path(pa, p, bcasts, ctxn):
            """Transposed scores -> exp -> context (per-head psum)."""
            for j in range(4):
                pctx = [pp.tile([64, 512], F32, name=f"pctx{ih}",
                                tag=f"ctx{ih}", bufs=1) for ih in range(2)]
                nki = (4 * j + 4) if causal else NT
                for ki in range(nki):
                    col0 = max(0, 128 * ki - 512 * j) if causal else 0
                    diag = causal and (128 * ki >= 512 * j)
                    st_ = pp.tile([128, 1024], F32, name="st", tag="sa",
                                  bufs=2)
                    ex_ = pa.tile([128, 1024], R32, name="ex", tag="ex",
                                  bufs=3)
                    for ih, hl in enumerate((2 * p, 2 * p + 1)):
                        base = ih * 512
                        ro = (hl % 2) * 64
                        nc.tensor.matmul(
                            st_[:, base + col0:base + 512],
                            kt[p][ro:ro + 64, ki * 128:ki * 128 + 128],
                            qt[p][ro:ro + 64, j * 512 + col0:j * 512 + 512],
                            start=True, stop=not diag)
                        if diag:
                            nc.tensor.matmul(
                                st_[:, base + col0:base + col0 + 128],
                                idnr[:], negt[:], start=False, stop=True)
                    nc.scalar.activation(
                        ex_[:, col0:1024], st_[:, col0:1024], AFT.Exp,
                        scale=0.125, bias=padb[:, ki:ki + 1])
                    for ih, hl in enumerate((2 * p, 2 * p + 1)):
                        base = ih * 512
                        nc.tensor.matmul(
                            pctx[ih][0:64, col0:512],
                            vsb[:, ki * 256 + hl * 64:ki * 256 + hl * 64 + 64],
                            ex_[:, base + col0:base + 512],
                            start=(ki == 0), stop=(ki == nki - 1))
                for ih, hl in enumerate((2 * p, 2 * p + 1)):
                    nc.vector.tensor_tensor(
                        out=ctxn[hl][:, j * 512:j * 512 + 512],
                        in0=pctx[ih][0:64, :],
                        in1=bcasts[ih][:, j * 512:j * 512 + 512],
                        op=mybir.AluOpType.mult)

        def attn_phase(pa):
            rsum = pa.tile([128, 64], F32, name="rsum")
            sc2 = pa.tile([128, 2], F32, name="sc2", bufs=3)
            ctxn = [pa.tile([64, S], R32, name=f"ctxn{h}", tag="ctxn",
                            bufs=4) for h in range(4)]
            for p in range(2):
                for hl in (2 * p, 2 * p + 1):
                    nat_path(pa, p, hl, rsum, sc2)
                bcasts = [bcast_recip(pa, hl, rsum)
                          for hl in (2 * p, 2 * p + 1)]
                t_path(pa, p, bcasts, ctxn)
            # output projection: partial out rows, 4-head accumulation
            for st in range(NT):
                outst = pa.tile([128, 1024], F32, name="outst", tag="outst",
                                bufs=2)
                for oc in range(2):
                    po_ = pp.tile([128, 512], F32, name="po", tag="mm",
                                  bufs=2)
                    for hl in range(4):
                        nc.tensor.matmul(
                            po_[:], ctxn[hl][:, st * 128:st * 128 + 128],
                            wo[:, hl * 1024 + oc * 512:
                               hl * 1024 + oc * 512 + 512],
                            start=(hl == 0), stop=(hl == 3))
                    nc.vector.tensor_copy(
                        out=outst[:, oc * 512:oc * 512 + 512], in_=po_[:])
                nc.sync.dma_start(out=OUT_[st * 128:st * 128 + 128, :],
                                  in_=outst[:])

        def body():
            load_consts()
            with tc.tile_pool(name="px", bufs=1) as px:
                qkv_phase(px)
            with tc.tile_pool(name="pa", bufs=1) as pa:
                attn_phase(pa)
            if timing:
                prb = pw.tile([128, 16], F32, name="prb")
                nc.vector.memset(prb[:], 1.0)
                nc.sync.dma_start(out=PRB_[:], in_=prb[:])

        if timing and nreps > 1:
            with tc.For_i(0, nreps, 1):
                body()
        else:
            body()
        est.close()
    nc.finalize()
    return nc


def _get(causal, padded, timing=False, nreps=1):
    key = (causal, padded, timing, nreps)
    if key not in _BUILD_CACHE:
        _BUILD_CACHE[key] = _build(causal, padded, timing, nreps)
    return _BUILD_CACHE[key]


def host_prep(X, Wq, bq, Wk, bk, Wv, bv, Wo, bo, padding_mask, causal_mask):
    X = np.asarray(X, dtype=np.float32)
    padding_mask = np.asarray(padding_mask)
    causal = bool(np.asarray(causal_mask).item())
    padded = bool(np.any(padding_mask == 1))

    neg = np.triu(np.full((128, 128), NEGV, np.float32), k=1)
    negt = np.tril(np.full((128, 128), NEGV, np.float32), k=-1)
    idn = np.eye(128, dtype=np.float32)
    in_maps = []
    for c in range(8):
        b, g = c // 4, c % 4
        hs = slice(256 * g, 256 * (g + 1))
        padv = (padding_mask[b] == 1).astype(np.float32) * (8.0 * NEGV)
        m = {
            "XT": np.ascontiguousarray(X[b].T),
            "WQT": np.ascontiguousarray(np.asarray(Wq, dtype=np.float32)[hs, :].T),
            "WKT": np.ascontiguousarray(np.asarray(Wk, dtype=np.float32)[hs, :].T),
            "WVT": np.ascontiguousarray(np.asarray(Wv, dtype=np.float32)[hs, :].T),
            "WOT": np.ascontiguousarray(np.asarray(Wo, dtype=np.float32)[:, hs].T),
            "BIA": np.concatenate([
                np.asarray(bq, dtype=np.float32)[hs],
                np.asarray(bk, dtype=np.float32)[hs],
                np.asarray(bv, dtype=np.float32)[hs]]).reshape(1, 768),
            "PADB": np.ascontiguousarray(padv.reshape(NT, 128).T),
            "PADR": (padv / 8.0).reshape(1, S).astype(np.float32),
            "NEG": neg,
            "NEGT": negt,
            "IDNR": idn,
            "IDN32": idn,
        }
        in_maps.append(m)
    return in_maps, causal, padded


def kernel(X, Wq, bq, Wk, bk, Wv, bv, Wo, bo, padding_mask, causal_mask):
    in_maps, causal, padded = host_prep(
        X, Wq, bq, Wk, bk, Wv, bv, Wo, bo, padding_mask, causal_mask)
    nc = _get(causal, padded)
    res = run_bass_kernel_spmd(nc, in_maps, list(range(8)))
    out = np.zeros((B, S, D), np.float32)
    alpha = np.zeros((B, H, S, S), np.float32)
    for c in range(8):
        b, g = c // 4, c % 4
        out[b] += res.results[c]["OUT"]
        alpha[b, 4 * g:4 * g + 4] = res.results[c]["ALP"]
    out += np.asarray(bo, dtype=np.float32)[None, None, :]
    return out, alpha


# revision 8
# speedup vs baseline: 2.5905x; 2.5905x over previous
"""Bass/Trainium2 kernel for nn_MultiHeadAttention (B=2,S=2048,D=1024,H=16).

Sharding: 8 cores = 2 batches x 4 head-groups (Megatron-style). Each core
computes 4 heads of one batch: QKV projections from X^T (replicated per
batch), causal softmax (alpha written to HBM), context, and a partial
output projection. Host sums the 4 partial outputs per batch (the W_o
all-reduce) and concatenates alpha shards. The causal upper triangle of
alpha is never written on device: output buffers are pre-zeroed.

All matmuls run in float32r (full-rate on the PE at N>=256, ~1e-4 rel
error); accumulation and softmax are fp32.
"""
import numpy as np

import concourse.bass as bass
import concourse.mybir as mybir
from concourse import bacc
from concourse.tile import TileContext
from concourse.bass_utils import run_bass_kernel_spmd

F32 = mybir.dt.float32
R32 = mybir.dt.float32r
AFT = mybir.ActivationFunctionType

B, S, D, H = 2, 2048, 1024, 16
DK = D // H          # 64
HPC = 4              # heads per core
NT = S // 128        # 16 q/k tiles
NEGV = -3.0e37

_BUILD_CACHE = {}


def _build(causal: bool, padded: bool, timing: bool = False, nreps: int = 1):
    nc = bacc.Bacc()
    XT_ = nc.declare_dram_parameter("XT", [D, S], R32, isOutput=False)
    WQT_ = nc.declare_dram_parameter("WQT", [D, 256], R32, isOutput=False)
    WKT_ = nc.declare_dram_parameter("WKT", [D, 256], R32, isOutput=False)
    WVT_ = nc.declare_dram_parameter("WVT", [D, 256], R32, isOutput=False)
    WOT_ = nc.declare_dram_parameter("WOT", [256, D], R32, isOutput=False)
    BIA_ = nc.declare_dram_parameter("BIA", [1, 768], R32, isOutput=False)
    PADB_ = nc.declare_dram_parameter("PADB", [128, NT], F32, isOutput=False)
    PADR_ = nc.declare_dram_parameter("PADR", [1, S], R32, isOutput=False)
    NEG_ = nc.declare_dram_parameter("NEG", [128, 128], R32, isOutput=False)
    NEGT_ = nc.declare_dram_parameter("NEGT", [128, 128], R32, isOutput=False)
    IDNR_ = nc.declare_dram_parameter("IDNR", [128, 128], R32, isOutput=False)
    IDN32_ = nc.declare_dram_parameter("IDN32", [128, 128], F32, isOutput=False)
    if timing:
        ALP_ = nc.dram_tensor("alp_i", [HPC, S, S], F32)
        OUT_ = nc.dram_tensor("out_i", [S, D], F32)
        PRB_ = nc.declare_dram_parameter("PRB", [128, 16], F32, isOutput=True)
    else:
        ALP_ = nc.declare_dram_parameter("ALP", [HPC, S, S], F32, isOutput=True)
        OUT_ = nc.declare_dram_parameter("OUT", [S, D], F32, isOutput=True)
        PRB_ = None

    with TileContext(nc) as tc:
        import contextlib
        est = contextlib.ExitStack()
        pw = est.enter_context(tc.tile_pool(name="pw", bufs=1))
        pp = est.enter_context(tc.tile_pool(name="pp", bufs=1, space="PSUM"))

        # ---- persistent tiles (live through attention) -------------------
        wo = pw.tile([64, 4 * 1024], R32, name="wo")
        padb = pw.tile([128, NT], F32, name="padb")
        neg = pw.tile([128, 128], R32, name="neg")
        negt = pw.tile([128, 128], R32, name="negt")
        idnr = pw.tile([128, 128], R32, name="idnr")
        idn32 = pw.tile([128, 128], F32, name="idn32")
        ones32 = pw.tile([1, 512], F32, name="ones32")
        onesr = pw.tile([1, 512], R32, name="onesr")
        padr = pw.tile([1, S], R32, name="padr") if padded else None
        qt = [pw.tile([128, S], R32, name=f"qt{p}") for p in range(2)]
        kt = [pw.tile([128, S], R32, name=f"kt{p}") for p in range(2)]
        vsb = pw.tile([128, NT * 256], R32, name="vsb")

        def rearr8(ap):
            return ap.rearrange("(t p) c -> p t c", p=128)

        def load_consts():
            nc.sync.dma_start(out=wo[:].rearrange("p (t c) -> p t c", t=4),
                              in_=WOT_[:].rearrange("(t p) c -> p t c", p=64))
            nc.sync.dma_start(out=padb[:], in_=PADB_[:])
            if padded:
                nc.sync.dma_start(out=padr[:], in_=PADR_[:])
            nc.sync.dma_start(out=neg[:], in_=NEG_[:])
            nc.sync.dma_start(out=negt[:], in_=NEGT_[:])
            nc.sync.dma_start(out=idnr[:], in_=IDNR_[:])
            nc.sync.dma_start(out=idn32[:], in_=IDN32_[:])
            nc.vector.memset(ones32[:], 1.0)
            nc.vector.tensor_copy(out=onesr[:], in_=ones32[:])

        def qkv_phase(px):
            wq = px.tile([128, 8 * 256], R32, name="wq")
            wk = px.tile([128, 8 * 256], R32, name="wk")
            wv = px.tile([128, 8 * 256], R32, name="wv")
            bia = px.tile([1, 768], R32, name="bia")
            xt = px.tile([128, 8 * S], R32, name="xt")
            nc.sync.dma_start(out=wq[:].rearrange("p (t c) -> p t c", t=8),
                              in_=rearr8(WQT_[:]))
            nc.sync.dma_start(out=wk[:].rearrange("p (t c) -> p t c", t=8),
                              in_=rearr8(WKT_[:]))
            nc.sync.dma_start(out=wv[:].rearrange("p (t c) -> p t c", t=8),
                              in_=rearr8(WVT_[:]))
            nc.sync.dma_start(out=bia[:], in_=BIA_[:])
            nc.sync.dma_start(out=xt[:].rearrange("p (t c) -> p t c", t=8),
                              in_=rearr8(XT_[:]))

            def xtt(t):
                return xt[:, t * S:(t + 1) * S]

            # QT/KT: stationary = weight block, moving = X^T s-chunk
            for (w, dst, bofs) in ((wq, qt, 0), (wk, kt, 256)):
                for p in range(2):
                    for sc in range(4):
                        ps_ = pp.tile([128, 512], F32, name="mmq", tag="mm",
                                      bufs=2)
                        for t in range(8):
                            nc.tensor.matmul(
                                ps_[:],
                                w[:, t * 256 + p * 128:t * 256 + p * 128 + 128],
                                xtt(t)[:, sc * 512:sc * 512 + 512],
                                start=(t == 0), stop=False)
                        nc.tensor.matmul(
                            ps_[:],
                            bia[0:1, bofs + p * 128:bofs + p * 128 + 128],
                            onesr[:], start=False, stop=True)
                        nc.vector.tensor_copy(
                            out=dst[p][:, sc * 512:sc * 512 + 512], in_=ps_[:])
            # V natural [s, dk_local]
            for st in range(NT):
                ps_ = pp.tile([128, 512], F32, name="mmv", tag="mm", bufs=2)
                for t in range(8):
                    nc.tensor.matmul(
                        ps_[:, 0:256],
                        xtt(t)[:, st * 128:st * 128 + 128],
                        wv[:, t * 256:t * 256 + 256],
                        start=(t == 0), stop=False)
                nc.tensor.matmul(ps_[:, 0:256], onesr[0:1, 0:128],
                                 bia[0:1, 512:768], start=False, stop=True)
                nc.vector.tensor_copy(out=vsb[:, st * 256:st * 256 + 256],
                                      in_=ps_[:, 0:256])

        def nat_path(pa, p, hl, rsum, sc2):
            """Natural-layout scores -> exp(+rowsum) -> normalize -> DMA."""
            ro = (hl % 2) * 64
            for qi in range(NT):
                wf = (qi + 1) * 128 if causal else S
                ntile = (wf + 1023) // 1024
                rowt = []
                for ti in range(ntile):
                    w0 = ti * 1024
                    w1 = min(wf, w0 + 1024)
                    ps_ = pp.tile([128, 1024], F32, name="sa", tag="sa",
                                  bufs=2)
                    rowt.append((ps_, w0, w1))
                    for c0 in range(w0, w1, 512):
                        c1 = min(w1, c0 + 512)
                        has_neg = causal and (c1 == wf)
                        nc.tensor.matmul(
                            ps_[:, c0 - w0:c1 - w0],
                            qt[p][ro:ro + 64, qi * 128:qi * 128 + 128],
                            kt[p][ro:ro + 64, c0:c1],
                            start=True, stop=not padded and not has_neg)
                        if padded:
                            nc.tensor.matmul(
                                ps_[:, c0 - w0:c1 - w0],
                                onesr[0:1, 0:128], padr[0:1, c0:c1],
                                start=False, stop=not has_neg)
                        if has_neg:
                            nc.tensor.matmul(
                                ps_[:, wf - 128 - w0:wf - w0],
                                idnr[:], neg[:], start=False, stop=True)
                # exp with free row-sum accumulation
                alb = pa.tile([128, S], F32, name="alb", tag="alb", bufs=2)
                col = hl * 16 + qi
                for ti, (ps_, w0, w1) in enumerate(rowt):
                    acc = (rsum[:, col:col + 1] if ntile == 1
                           else sc2[:, ti:ti + 1])
                    nc.scalar.activation(
                        alb[:, w0:w1], ps_[:, 0:w1 - w0], AFT.Exp,
                        scale=0.125, accum_out=acc)
                if ntile > 1:
                    nc.vector.tensor_tensor(
                        out=rsum[:, col:col + 1], in0=sc2[:, 0:1],
                        in1=sc2[:, 1:2], op=mybir.AluOpType.add)
                rc = pa.tile([128, 1], F32, name="rc", tag="rc", bufs=3)
                nc.vector.reciprocal(out=rc[:], in_=rsum[:, col:col + 1])
                nc.vector.tensor_scalar_mul(alb[:, 0:wf], alb[:, 0:wf],
                                            rc[:, 0:1])
                nc.sync.dma_start(
                    out=ALP_[hl, qi * 128:qi * 128 + 128, 0:wf],
                    in_=alb[:, 0:wf])

        def bcast_recip(pa, hl, rsum):
            """[64, S] broadcast tile of 1/rowsum for one head."""
            bcast = pa.tile([64, S], F32, name="bcast", tag="bcast", bufs=2)
            pt_ = pp.tile([128, 512], F32, name="ptr", tag="mm", bufs=2)
            nc.tensor.transpose(pt_[0:16, 0:128],
                                rsum[:, hl * 16:hl * 16 + 16], idn32[:])
            rrt = pa.tile([16, 128], R32, name="rrt", tag="rrt", bufs=2)
            with nc.allow_low_precision(reason="f32r recip feeds matmul"):
                nc.vector.reciprocal(out=rrt[:], in_=pt_[0:16, 0:128])
            row = pa.tile([1, S], R32, name="row", tag="row", bufs=2)
            nc.sync.dma_start(out=row[:], in_=rrt[:])
            for sc in range(4):
                pb_ = pp.tile([128, 512], F32, name="pb", tag="mm", bufs=2)
                nc.tensor.matmul(pb_[0:64, :], onesr[0:1, 0:64],
                                 row[0:1, sc * 512:sc * 512 + 512],
                                 start=True, stop=True)
                nc.vector.tensor_copy(
                    out=bcast[:, sc * 512:sc * 512 + 512], in_=pb_[0:64, :])
            return bcast

        def t_path(pa, p, bcasts, ctxn):
            """Transposed scores -> exp -> context (per-head psum)."""
            for j in range(4):
                pctx = [pp.tile([64, 512], F32, name=f"pctx{ih}",
                                tag=f"ctx{ih}", bufs=1) for ih in range(2)]
                nki = (4 * j + 4) if causal else NT
                for ki in range(nki):
                    col0 = max(0, 128 * ki - 512 * j) if causal else 0
                    diag = causal and (128 * ki >= 512 * j)
                    st_ = pp.tile([128, 1024], F32, name="st", tag="sa",
                                  bufs=2)
                    ex_ = pa.tile([128, 1024], R32, name="ex", tag="ex",
                                  bufs=3)
                    for ih, hl in enumerate((2 * p, 2 * p + 1)):
                        base = ih * 512
                        ro = (hl % 2) * 64
                        nc.tensor.matmul(
                            st_[:, base + col0:base + 512],
                            kt[p][ro:ro + 64, ki * 128:ki * 128 + 128],
                            qt[p][ro:ro + 64, j * 512 + col0:j * 512 + 512],
                            start=True, stop=not diag)
                        if diag:
                            nc.tensor.matmul(
                                st_[:, base + col0:base + col0 + 128],
                                idnr[:], negt[:], start=False, stop=True)
                    nc.scalar.activation(
                        ex_[:, col0:1024], st_[:, col0:1024], AFT.Exp,
                        scale=0.125, bias=padb[:, ki:ki + 1])
                    for ih, hl in enumerate((2 * p, 2 * p + 1)):
                        base = ih * 512
                        nc.tensor.matmul(
                            pctx[ih][0:64, col0:512],
                            vsb[:, ki * 256 + hl * 64:ki * 256 + hl * 64 + 64],
                            ex_[:, base + col0:base + 512],
                            start=(ki == 0), stop=(ki == nki - 1))
                for ih, hl in enumerate((2 * p, 2 * p + 1)):
                    nc.vector.tensor_tensor(
                        out=ctxn[hl][:, j * 512:j * 512 + 512],
                        in0=pctx[ih][0:64, :],
                        in1=bcasts[ih][:, j * 512:j * 512 + 512],
                        op=mybir.AluOpType.mult)

        def attn_phase(pa):
            rsum = pa.tile([128, 64], F32, name="rsum")
            sc2 = pa.tile([128, 2], F32, name="sc2", bufs=3)
            ctxn = [pa.tile([64, S], R32, name=f"ctxn{h}", tag="ctxn",
                            bufs=4) for h in range(4)]
            for p in range(2):
                for hl in (2 * p, 2 * p + 1):
                    nat_path(pa, p, hl, rsum, sc2)
                bcasts = [bcast_recip(pa, hl, rsum)
                          for hl in (2 * p, 2 * p + 1)]
                t_path(pa, p, bcasts, ctxn)
            # output projection: partial out rows, 4-head accumulation
            for st in range(NT):
                outst = pa.tile([128, 1024], F32, name="outst", tag="outst",
                                bufs=2)
                for oc in range(2):
                    po_ = pp.tile([128, 512], F32, name="po", tag="mm",
                                  bufs=2)
                    for hl in range(4):
                        nc.tensor.matmul(
                            po_[:], ctxn[hl][:, st * 128:st * 128 + 128],
                            wo[:, hl * 1024 + oc * 512:
                               hl * 1024 + oc * 512 + 512],
                            start=(hl == 0), stop=(hl == 3))
                    nc.vector.tensor_copy(
                        out=outst[:, oc * 512:oc * 512 + 512], in_=po_[:])
                nc.sync.dma_start(out=OUT_[st * 128:st * 128 + 128, :],
                                  in_=outst[:])

        def body():
            load_consts()
            with tc.tile_pool(name="px", bufs=1) as px:
                qkv_phase(px)
            with tc.tile_pool(name="pa", bufs=1) as pa:
                attn_phase(pa)
            if timing:
                prb = pw.tile([128, 16], F32, name="prb")
                nc.vector.memset(prb[:], 1.0)
                nc.sync.dma_start(out=PRB_[:], in_=prb[:])

        if timing and nreps > 1:
            with tc.For_i(0, nreps, 1):
                body()
        else:
            body()
        est.close()
    nc.finalize()
    return nc


def _get(causal, padded, timing=False, nreps=1):
    key = (causal, padded, timing, nreps)
    if key not in _BUILD_CACHE:
        _BUILD_CACHE[key] = _build(causal, padded, timing, nreps)
    return _BUILD_CACHE[key]


def host_prep(X, Wq, bq, Wk, bk, Wv, bv, Wo, bo, padding_mask, causal_mask):
    X = np.asarray(X, dtype=np.float32)
    padding_mask = np.asarray(padding_mask)
    causal = bool(np.asarray(causal_mask).item())
    padded = bool(np.any(padding_mask == 1))

    neg = np.triu(np.full((128, 128), NEGV, np.float32), k=1)
    negt = np.tril(np.full((128, 128), NEGV, np.float32), k=-1)
    idn = np.eye(128, dtype=np.float32)
    in_maps = []
    for c in range(8):
        b, g = c // 4, c % 4
        hs = slice(256 * g, 256 * (g + 1))
        padv = (padding_mask[b] == 1).astype(np.float32) * (8.0 * NEGV)
        m = {
            "XT": np.ascontiguousarray(X[b].T),
            "WQT": np.ascontiguousarray(np.asarray(Wq, dtype=np.float32)[hs, :].T),
            "WKT": np.ascontiguousarray(np.asarray(Wk, dtype=np.float32)[hs, :].T),
            "WVT": np.ascontiguousarray(np.asarray(Wv, dtype=np.float32)[hs, :].T),
            "WOT": np.ascontiguousarray(np.asarray(Wo, dtype=np.float32)[:, hs].T),
            "BIA": np.concatenate([
                np.asarray(bq, dtype=np.float32)[hs],
                np.asarray(bk, dtype=np.float32)[hs],
                np.asarray(bv, dtype=np.float32)[hs]]).reshape(1, 768),
            "PADB": np.ascontiguousarray(padv.reshape(NT, 128).T),
            "PADR": (padv / 8.0).reshape(1, S).astype(np.float32),
            "NEG": neg,
            "NEGT": negt,
            "IDNR": idn,
            "IDN32": idn,
        }
        in_maps.append(m)
    return in_maps, causal, padded


def kernel(X, Wq, bq, Wk, bk, Wv, bv, Wo, bo, padding_mask, causal_mask):
    in_maps, causal, padded = host_prep(
        X, Wq, bq, Wk, bk, Wv, bv, Wo, bo, padding_mask, causal_mask)
    nc = _get(causal, padded)
    res = run_bass_kernel_spmd(nc, in_maps, list(range(8)))
    out = np.zeros((B, S, D), np.float32)
    alpha = np.zeros((B, H, S, S), np.float32)
    for c in range(8):
        b, g = c // 4, c % 4
        out[b] += res.results[c]["OUT"]
        alpha[b, 4 * g:4 * g + 4] = res.results[c]["ALP"]
    out += np.asarray(bo, dtype=np.float32)[None, None, :]
    return out, alpha


# revision 9
# speedup vs baseline: 2.8159x; 1.0870x over previous
"""Bass/Trainium2 kernel for nn_MultiHeadAttention (B=2,S=2048,D=1024,H=16).

Sharding: 8 cores = 2 batches x 4 head-groups (Megatron-style). Each core
computes 4 heads of one batch: QKV projections from X^T (replicated per
batch), causal softmax (alpha written to HBM), context, and a partial
output projection. Host sums the 4 partial outputs per batch (the W_o
all-reduce) and concatenates alpha shards. The causal upper triangle of
alpha is never written on device: output buffers are pre-zeroed.

All matmuls run in float32r (full-rate on the PE at N>=256, ~1e-4 rel
error); accumulation and softmax are fp32.
"""
import numpy as np

import concourse.bass as bass
import concourse.mybir as mybir
from concourse import bacc
from concourse.tile import TileContext
from concourse.bass_utils import run_bass_kernel_spmd

F32 = mybir.dt.float32
R32 = mybir.dt.float32r
AFT = mybir.ActivationFunctionType

B, S, D, H = 2, 2048, 1024, 16
DK = D // H          # 64
HPC = 4              # heads per core
NT = S // 128        # 16 q/k tiles
NEGV = -3.0e37

_BUILD_CACHE = {}


def _build(causal: bool, padded: bool, timing: bool = False, nreps: int = 1):
    nc = bacc.Bacc()
    XT_ = nc.declare_dram_parameter("XT", [D, S], R32, isOutput=False)
    WQT_ = nc.declare_dram_parameter("WQT", [D, 256], R32, isOutput=False)
    WKT_ = nc.declare_dram_parameter("WKT", [D, 256], R32, isOutput=False)
    WVT_ = nc.declare_dram_parameter("WVT", [D, 256], R32, isOutput=False)
    WOT_ = nc.declare_dram_parameter("WOT", [256, D], R32, isOutput=False)
    BIA_ = nc.declare_dram_parameter("BIA", [1, 768], R32, isOutput=False)
    PADB_ = nc.declare_dram_parameter("PADB", [128, NT], F32, isOutput=False)
    PADR_ = nc.declare_dram_parameter("PADR", [1, S], R32, isOutput=False)
    NEG_ = nc.declare_dram_parameter("NEG", [128, 128], R32, isOutput=False)
    NEGT_ = nc.declare_dram_parameter("NEGT", [128, 128], R32, isOutput=False)
    IDNR_ = nc.declare_dram_parameter("IDNR", [128, 128], R32, isOutput=False)
    IDN32_ = nc.declare_dram_parameter("IDN32", [128, 128], F32, isOutput=False)
    if timing:
        ALP_ = nc.dram_tensor("alp_i", [HPC, S, S], F32)
        OUT_ = nc.dram_tensor("out_i", [S, D], F32)
        PRB_ = nc.declare_dram_parameter("PRB", [128, 16], F32, isOutput=True)
    else:
        ALP_ = nc.declare_dram_parameter("ALP", [HPC, S, S], F32, isOutput=True)
        OUT_ = nc.declare_dram_parameter("OUT", [S, D], F32, isOutput=True)
        PRB_ = None

    with TileContext(nc) as tc:
        import contextlib
        est = contextlib.ExitStack()
        pw = est.enter_context(tc.tile_pool(name="pw", bufs=1))
        pp = est.enter_context(tc.tile_pool(name="pp", bufs=1, space="PSUM"))

        # ---- persistent tiles (live through attention) -------------------
        wo = pw.tile([64, 4 * 1024], R32, name="wo")
        padb = pw.tile([128, NT], F32, name="padb")
        neg = pw.tile([128, 128], R32, name="neg")
        negt = pw.tile([128, 128], R32, name="negt")
        idnr = pw.tile([128, 128], R32, name="idnr")
        idn32 = pw.tile([128, 128], F32, name="idn32")
        ones32 = pw.tile([1, 512], F32, name="ones32")
        onesr = pw.tile([1, 512], R32, name="onesr")
        padr = pw.tile([1, S], R32, name="padr") if padded else None
        qt = [pw.tile([128, S], R32, name=f"qt{p}") for p in range(2)]
        kt = [pw.tile([128, S], R32, name=f"kt{p}") for p in range(2)]
        vsb = pw.tile([128, NT * 256], R32, name="vsb")

        def rearr8(ap):
            return ap.rearrange("(t p) c -> p t c", p=128)

        def load_consts():
            nc.sync.dma_start(out=wo[:].rearrange("p (t c) -> p t c", t=4),
                              in_=WOT_[:].rearrange("(t p) c -> p t c", p=64))
            nc.sync.dma_start(out=padb[:], in_=PADB_[:])
            if padded:
                nc.sync.dma_start(out=padr[:], in_=PADR_[:])
            nc.sync.dma_start(out=neg[:], in_=NEG_[:])
            nc.sync.dma_start(out=negt[:], in_=NEGT_[:])
            nc.sync.dma_start(out=idnr[:], in_=IDNR_[:])
            nc.sync.dma_start(out=idn32[:], in_=IDN32_[:])
            nc.vector.memset(ones32[:], 1.0)
            nc.vector.tensor_copy(out=onesr[:], in_=ones32[:])

        def qkv_phase(px):
            wq = px.tile([128, 8 * 256], R32, name="wq")
            wk = px.tile([128, 8 * 256], R32, name="wk")
            wv = px.tile([128, 8 * 256], R32, name="wv")
            bia = px.tile([1, 768], R32, name="bia")
            xt = px.tile([128, 8 * S], R32, name="xt")
            nc.sync.dma_start(out=wq[:].rearrange("p (t c) -> p t c", t=8),
                              in_=rearr8(WQT_[:]))
            nc.sync.dma_start(out=wk[:].rearrange("p (t c) -> p t c", t=8),
                              in_=rearr8(WKT_[:]))
            nc.sync.dma_start(out=wv[:].rearrange("p (t c) -> p t c", t=8),
                              in_=rearr8(WVT_[:]))
            nc.sync.dma_start(out=bia[:], in_=BIA_[:])
            nc.sync.dma_start(out=xt[:].rearrange("p (t c) -> p t c", t=8),
                              in_=rearr8(XT_[:]))

            def xtt(t):
                return xt[:, t * S:(t + 1) * S]

            # QT/KT: stationary = weight block, moving = X^T s-chunk
            for (w, dst, bofs) in ((wq, qt, 0), (wk, kt, 256)):
                for p in range(2):
                    for sc in range(4):
                        ps_ = pp.tile([128, 512], F32, name="mmq", tag="mm",
                                      bufs=2)
                        for t in range(8):
                            nc.tensor.matmul(
                                ps_[:],
                                w[:, t * 256 + p * 128:t * 256 + p * 128 + 128],
                                xtt(t)[:, sc * 512:sc * 512 + 512],
                                start=(t == 0), stop=False)
                        nc.tensor.matmul(
                            ps_[:],
                            bia[0:1, bofs + p * 128:bofs + p * 128 + 128],
                            onesr[:], start=False, stop=True)
                        nc.vector.tensor_copy(
                            out=dst[p][:, sc * 512:sc * 512 + 512], in_=ps_[:])
            # V natural [s, dk_local]
            for st in range(NT):
                ps_ = pp.tile([128, 512], F32, name="mmv", tag="mm", bufs=2)
                for t in range(8):
                    nc.tensor.matmul(
                        ps_[:, 0:256],
                        xtt(t)[:, st * 128:st * 128 + 128],
                        wv[:, t * 256:t * 256 + 256],
                        start=(t == 0), stop=False)
                nc.tensor.matmul(ps_[:, 0:256], onesr[0:1, 0:128],
                                 bia[0:1, 512:768], start=False, stop=True)
                nc.vector.tensor_copy(out=vsb[:, st * 256:st * 256 + 256],
                                      in_=ps_[:, 0:256])

        def nat_path(pa, p, hl, rsum, sc2):
            """Natural-layout scores -> exp(+rowsum) -> normalize -> DMA."""
            ro = (hl % 2) * 64
            for qi in range(NT):
                wf = (qi + 1) * 128 if causal else S
                ntile = (wf + 1023) // 1024
                rowt = []
                for ti in range(ntile):
                    w0 = ti * 1024
                    w1 = min(wf, w0 + 1024)
                    ps_ = pp.tile([128, 1024], F32, name="sa", tag="sa",
                                  bufs=2)
                    rowt.append((ps_, w0, w1))
                    for c0 in range(w0, w1, 512):
                        c1 = min(w1, c0 + 512)
                        has_neg = causal and (c1 == wf)
                        nc.tensor.matmul(
                            ps_[:, c0 - w0:c1 - w0],
                            qt[p][ro:ro + 64, qi * 128:qi * 128 + 128],
                            kt[p][ro:ro + 64, c0:c1],
                            start=True, stop=not padded and not has_neg)
                        if padded:
                            nc.tensor.matmul(
                                ps_[:, c0 - w0:c1 - w0],
                                onesr[0:1, 0:128], padr[0:1, c0:c1],
                                start=False, stop=not has_neg)
                        if has_neg:
                            nc.tensor.matmul(
                                ps_[:, wf - 128 - w0:wf - w0],
                                idnr[:], neg[:], start=False, stop=True)
                # exp with free row-sum accumulation
                alb = pa.tile([128, S], F32, name="alb", tag="alb", bufs=3)
                col = hl * 16 + qi
                for ti, (ps_, w0, w1) in enumerate(rowt):
                    acc = (rsum[:, col:col + 1] if ntile == 1
                           else sc2[:, ti:ti + 1])
                    nc.scalar.activation(
                        alb[:, w0:w1], ps_[:, 0:w1 - w0], AFT.Exp,
                        scale=0.125, accum_out=acc)
                if ntile > 1:
                    nc.vector.tensor_tensor(
                        out=rsum[:, col:col + 1], in0=sc2[:, 0:1],
                        in1=sc2[:, 1:2], op=mybir.AluOpType.add)
                rc = pa.tile([128, 1], F32, name="rc", tag="rc", bufs=3)
                nc.vector.reciprocal(out=rc[:], in_=rsum[:, col:col + 1])
                nc.vector.tensor_scalar_mul(alb[:, 0:wf], alb[:, 0:wf],
                                            rc[:, 0:1])
                nc.sync.dma_start(
                    out=ALP_[hl, qi * 128:qi * 128 + 128, 0:wf],
                    in_=alb[:, 0:wf])

        def bcast_recip(pa, hl, rsum):
            """[64, S] broadcast tile of 1/rowsum for one head."""
            bcast = pa.tile([64, S], F32, name="bcast", tag="bcast", bufs=2)
            pt_ = pp.tile([128, 512], F32, name="ptr", tag="mm", bufs=2)
            nc.tensor.transpose(pt_[0:16, 0:128],
                                rsum[:, hl * 16:hl * 16 + 16], idn32[:])
            rrt = pa.tile([16, 128], R32, name="rrt", tag="rrt", bufs=2)
            with nc.allow_low_precision(reason="f32r recip feeds matmul"):
                nc.vector.reciprocal(out=rrt[:], in_=pt_[0:16, 0:128])
            row = pa.tile([1, S], R32, name="row", tag="row", bufs=2)
            nc.sync.dma_start(out=row[:], in_=rrt[:])
            for sc in range(4):
                pb_ = pp.tile([128, 512], F32, name="pb", tag="mm", bufs=2)
                nc.tensor.matmul(pb_[0:64, :], onesr[0:1, 0:64],
                                 row[0:1, sc * 512:sc * 512 + 512],
                                 start=True, stop=True)
                nc.vector.tensor_copy(
                    out=bcast[:, sc * 512:sc * 512 + 512], in_=pb_[0:64, :])
            return bcast

        def t_path(pa, p, bcasts, ctxn):
            """Transposed scores -> exp -> context (per-head psum)."""
            for j in range(4):
                pctx = [pp.tile([64, 512], F32, name=f"pctx{ih}",
                                tag=f"ctx{ih}", bufs=1) for ih in range(2)]
                nki = (4 * j + 4) if causal else NT
                for ki in range(nki):
                    col0 = max(0, 128 * ki - 512 * j) if causal else 0
                    diag = causal and (128 * ki >= 512 * j)
                    st_ = pp.tile([128, 1024], F32, name="st", tag="sa",
                                  bufs=2)
                    ex_ = pa.tile([128, 1024], R32, name="ex", tag="ex",
                                  bufs=4)
                    for ih, hl in enumerate((2 * p, 2 * p + 1)):
                        base = ih * 512
                        ro = (hl % 2) * 64
                        nc.tensor.matmul(
                            st_[:, base + col0:base + 512],
                            kt[p][ro:ro + 64, ki * 128:ki * 128 + 128],
                            qt[p][ro:ro + 64, j * 512 + col0:j * 512 + 512],
                            start=True, stop=not diag)
                        if diag:
                            nc.tensor.matmul(
                                st_[:, base + col0:base + col0 + 128],
                                idnr[:], negt[:], start=False, stop=True)
                    nc.scalar.activation(
                        ex_[:, col0:1024], st_[:, col0:1024], AFT.Exp,
                        scale=0.125, bias=padb[:, ki:ki + 1])
                    for ih, hl in enumerate((2 * p, 2 * p + 1)):
                        base = ih * 512
                        nc.tensor.matmul(
                            pctx[ih][0:64, col0:512],
                            vsb[:, ki * 256 + hl * 64:ki * 256 + hl * 64 + 64],
                            ex_[:, base + col0:base + 512],
                            start=(ki == 0), stop=(ki == nki - 1))
                for ih, hl in enumerate((2 * p, 2 * p + 1)):
                    nc.vector.tensor_tensor(
                        out=ctxn[hl][:, j * 512:j * 512 + 512],
                        in0=pctx[ih][0:64, :],
                        in1=bcasts[ih][:, j * 512:j * 512 + 512],
                        op=mybir.AluOpType.mult)

        def attn_phase(pa):
            rsum = pa.tile([128, 64], F32, name="rsum")
            sc2 = pa.tile([128, 2], F32, name="sc2", bufs=3)
            ctxn = [pa.tile([64, S], R32, name=f"ctxn{h}", tag="ctxn",
                            bufs=4) for h in range(4)]
            for p in range(2):
                for hl in (2 * p, 2 * p + 1):
                    nat_path(pa, p, hl, rsum, sc2)
            for p in range(2):
                bcasts = [bcast_recip(pa, hl, rsum)
                          for hl in (2 * p, 2 * p + 1)]
                t_path(pa, p, bcasts, ctxn)
            # output projection: partial out rows, 4-head accumulation
            for st in range(NT):
                outst = pa.tile([128, 1024], F32, name="outst", tag="outst",
                                bufs=2)
                for oc in range(2):
                    po_ = pp.tile([128, 512], F32, name="po", tag="mm",
                                  bufs=2)
                    for hl in range(4):
                        nc.tensor.matmul(
                            po_[:], ctxn[hl][:, st * 128:st * 128 + 128],
                            wo[:, hl * 1024 + oc * 512:
                               hl * 1024 + oc * 512 + 512],
                            start=(hl == 0), stop=(hl == 3))
                    nc.vector.tensor_copy(
                        out=outst[:, oc * 512:oc * 512 + 512], in_=po_[:])
                nc.sync.dma_start(out=OUT_[st * 128:st * 128 + 128, :],
                                  in_=outst[:])

        def body():
            load_consts()
            with tc.tile_pool(name="px", bufs=1) as px:
                qkv_phase(px)
            with tc.tile_pool(name="pa", bufs=1) as pa:
                attn_phase(pa)
            if timing:
                prb = pw.tile([128, 16], F32, name="prb")
                nc.vector.memset(prb[:], 1.0)
                nc.sync.dma_start(out=PRB_[:], in_=prb[:])

        if timing and nreps > 1:
            with tc.For_i(0, nreps, 1):
                body()
        else:
            body()
        est.close()
    nc.finalize()
    return nc


def _get(causal, padded, timing=False, nreps=1):
    key = (causal, padded, timing, nreps)
    if key not in _BUILD_CACHE:
        _BUILD_CACHE[key] = _build(causal, padded, timing, nreps)
    return _BUILD_CACHE[key]


def host_prep(X, Wq, bq, Wk, bk, Wv, bv, Wo, bo, padding_mask, causal_mask):
    X = np.asarray(X, dtype=np.float32)
    padding_mask = np.asarray(padding_mask)
    causal = bool(np.asarray(causal_mask).item())
    padded = bool(np.any(padding_mask == 1))

    neg = np.triu(np.full((128, 128), NEGV, np.float32), k=1)
    negt = np.tril(np.full((128, 128), NEGV, np.float32), k=-1)
    idn = np.eye(128, dtype=np.float32)
    in_maps = []
    for c in range(8):
        b, g = c // 4, c % 4
        hs = slice(256 * g, 256 * (g + 1))
        padv = (padding_mask[b] == 1).astype(np.float32) * (8.0 * NEGV)
        m = {
            "XT": np.ascontiguousarray(X[b].T),
            "WQT": np.ascontiguousarray(np.asarray(Wq, dtype=np.float32)[hs, :].T),
            "WKT": np.ascontiguousarray(np.asarray(Wk, dtype=np.float32)[hs, :].T),
            "WVT": np.ascontiguousarray(np.asarray(Wv, dtype=np.float32)[hs, :].T),
            "WOT": np.ascontiguousarray(np.asarray(Wo, dtype=np.float32)[:, hs].T),
            "BIA": np.concatenate([
                np.asarray(bq, dtype=np.float32)[hs],
                np.asarray(bk, dtype=np.float32)[hs],
                np.asarray(bv, dtype=np.float32)[hs]]).reshape(1, 768),
            "PADB": np.ascontiguousarray(padv.reshape(NT, 128).T),
            "PADR": (padv / 8.0).reshape(1, S).astype(np.float32),
            "NEG": neg,
            "NEGT": negt,
            "IDNR": idn,
            "IDN32": idn,
        }
        in_maps.append(m)
    return in_maps, causal, padded


def kernel(X, Wq, bq, Wk, bk, Wv, bv, Wo, bo, padding_mask, causal_mask):
    in_maps, causal, padded = host_prep(
        X, Wq, bq, Wk, bk, Wv, bv, Wo, bo, padding_mask, causal_mask)
    nc = _get(causal, padded)
    res = run_bass_kernel_spmd(nc, in_maps, list(range(8)))
    out = np.zeros((B, S, D), np.float32)
    alpha = np.zeros((B, H, S, S), np.float32)
    for c in range(8):
        b, g = c // 4, c % 4
        out[b] += res.results[c]["OUT"]
        alpha[b, 4 * g:4 * g + 4] = res.results[c]["ALP"]
    out += np.asarray(bo, dtype=np.float32)[None, None, :]
    return out, alpha


# revision 10
# speedup vs baseline: 4.0589x; 1.4414x over previous
"""Bass/Trainium2 kernel for nn_MultiHeadAttention (B=2,S=2048,D=1024,H=16).

Sharding: 8 cores = 2 batches x 4 head-groups (Megatron-style). Each core
computes 4 heads of one batch: QKV projections from X^T (replicated per
batch), causal softmax (alpha written to HBM), context, and a partial
output projection. Host sums the 4 partial outputs per batch (the W_o
all-reduce) and concatenates alpha shards. The causal upper triangle of
alpha is never written on device: output buffers are pre-zeroed.

All matmuls run in float32r (full-rate on the PE at N>=256, ~1e-4 rel
error); accumulation and softmax are fp32.
"""
import numpy as np

import concourse.bass as bass
import concourse.mybir as mybir
from concourse import bacc
from concourse.tile import TileContext
from concourse.bass_utils import run_bass_kernel_spmd

F32 = mybir.dt.float32
R32 = mybir.dt.float32r
AFT = mybir.ActivationFunctionType

B, S, D, H = 2, 2048, 1024, 16
DK = D // H          # 64
HPC = 4              # heads per core
NT = S // 128        # 16 q/k tiles
NEGV = -3.0e37

_BUILD_CACHE = {}


def _build(causal: bool, padded: bool, biased: bool = False,
           timing: bool = False, nreps: int = 1):
    nc = bacc.Bacc()
    XT_ = nc.declare_dram_parameter("XT", [D, S], R32, isOutput=False)
    WQT_ = nc.declare_dram_parameter("WQT", [D, 256], R32, isOutput=False)
    WKT_ = nc.declare_dram_parameter("WKT", [D, 256], R32, isOutput=False)
    WVT_ = nc.declare_dram_parameter("WVT", [D, 256], R32, isOutput=False)
    WOT_ = nc.declare_dram_parameter("WOT", [256, D], R32, isOutput=False)
    BIA_ = nc.declare_dram_parameter("BIA", [1, 768], R32, isOutput=False)
    PADB_ = nc.declare_dram_parameter("PADB", [128, NT], F32, isOutput=False)
    PADR_ = nc.declare_dram_parameter("PADR", [1, S], R32, isOutput=False)
    NEG_ = nc.declare_dram_parameter("NEG", [128, 128], R32, isOutput=False)
    MSKT_ = nc.declare_dram_parameter("MSKT", [128, 128], F32, isOutput=False)
    IDNR_ = nc.declare_dram_parameter("IDNR", [128, 128], R32, isOutput=False)
    IDN32_ = nc.declare_dram_parameter("IDN32", [128, 128], F32, isOutput=False)
    if timing:
        ALP_ = nc.dram_tensor("alp_i", [HPC, S, S], F32)
        OUT_ = nc.dram_tensor("out_i", [S, D], F32)
        PRB_ = nc.declare_dram_parameter("PRB", [128, 16], F32, isOutput=True)
    else:
        ALP_ = nc.declare_dram_parameter("ALP", [HPC, S, S], F32, isOutput=True)
        OUT_ = nc.declare_dram_parameter("OUT", [S, D], F32, isOutput=True)
        PRB_ = None

    with TileContext(nc) as tc:
        import contextlib
        est = contextlib.ExitStack()
        pw = est.enter_context(tc.tile_pool(name="pw", bufs=1))
        pp = est.enter_context(tc.tile_pool(name="pp", bufs=1, space="PSUM"))

        # ---- persistent tiles (live through attention) -------------------
        wo = pw.tile([64, 4 * 1024], R32, name="wo")
        padb = pw.tile([128, NT], F32, name="padb")
        neg = pw.tile([128, 128], R32, name="neg")
        mskt = pw.tile([128, 128], F32, name="mskt")
        idnr = pw.tile([128, 128], R32, name="idnr")
        idn32 = pw.tile([128, 128], F32, name="idn32")
        ones32 = pw.tile([1, 512], F32, name="ones32")
        onesr = pw.tile([1, 512], R32, name="onesr")
        padr = pw.tile([1, S], R32, name="padr") if padded else None
        qt = [pw.tile([128, S], R32, name=f"qt{p}") for p in range(2)]
        kt = [pw.tile([128, S], R32, name=f"kt{p}") for p in range(2)]
        vsb = pw.tile([128, NT * 256], R32, name="vsb")

        def rearr8(ap):
            return ap.rearrange("(t p) c -> p t c", p=128)

        def load_consts():
            nc.sync.dma_start(out=wo[:].rearrange("p (t c) -> p t c", t=4),
                              in_=WOT_[:].rearrange("(t p) c -> p t c", p=64))
            nc.sync.dma_start(out=padb[:], in_=PADB_[:])
            if padded:
                nc.sync.dma_start(out=padr[:], in_=PADR_[:])
            nc.sync.dma_start(out=neg[:], in_=NEG_[:])
            nc.sync.dma_start(out=mskt[:], in_=MSKT_[:])
            nc.sync.dma_start(out=idnr[:], in_=IDNR_[:])
            nc.sync.dma_start(out=idn32[:], in_=IDN32_[:])
            nc.vector.memset(ones32[:], 1.0)
            nc.vector.tensor_copy(out=onesr[:], in_=ones32[:])

        def qkv_phase(px):
            wq = px.tile([128, 8 * 256], R32, name="wq")
            wk = px.tile([128, 8 * 256], R32, name="wk")
            wv = px.tile([128, 8 * 256], R32, name="wv")
            bia = px.tile([1, 768], R32, name="bia")
            xt = px.tile([128, 8 * S], R32, name="xt")
            nc.sync.dma_start(out=wq[:].rearrange("p (t c) -> p t c", t=8),
                              in_=rearr8(WQT_[:]))
            nc.sync.dma_start(out=wk[:].rearrange("p (t c) -> p t c", t=8),
                              in_=rearr8(WKT_[:]))
            nc.sync.dma_start(out=wv[:].rearrange("p (t c) -> p t c", t=8),
                              in_=rearr8(WVT_[:]))
            nc.sync.dma_start(out=bia[:], in_=BIA_[:])
            nc.sync.dma_start(out=xt[:].rearrange("p (t c) -> p t c", t=8),
                              in_=rearr8(XT_[:]))

            def xtt(t):
                return xt[:, t * S:(t + 1) * S]

            # QT/KT: stationary = weight block, moving = X^T s-chunk
            for (w, dst, bofs) in ((wq, qt, 0), (wk, kt, 256)):
                for p in range(2):
                    for sc in range(4):
                        ps_ = pp.tile([128, 512], F32, name="mmq", tag="mm",
                                      bufs=2)
                        for t in range(8):
                            nc.tensor.matmul(
                                ps_[:],
                                w[:, t * 256 + p * 128:t * 256 + p * 128 + 128],
                                xtt(t)[:, sc * 512:sc * 512 + 512],
                                start=(t == 0),
                                stop=(t == 7 and not biased))
                        if biased:
                            nc.tensor.matmul(
                                ps_[:],
                                bia[0:1, bofs + p * 128:bofs + p * 128 + 128],
                                onesr[:], start=False, stop=True)
                        nc.vector.tensor_copy(
                            out=dst[p][:, sc * 512:sc * 512 + 512], in_=ps_[:])
            # V natural [s, dk_local]
            for st in range(NT):
                ps_ = pp.tile([128, 512], F32, name="mmv", tag="mm", bufs=2)
                for t in range(8):
                    nc.tensor.matmul(
                        ps_[:, 0:256],
                        xtt(t)[:, st * 128:st * 128 + 128],
                        wv[:, t * 256:t * 256 + 256],
                        start=(t == 0), stop=(t == 7 and not biased))
                if biased:
                    nc.tensor.matmul(ps_[:, 0:256], onesr[0:1, 0:128],
                                     bia[0:1, 512:768], start=False,
                                     stop=True)
                nc.vector.tensor_copy(out=vsb[:, st * 256:st * 256 + 256],
                                      in_=ps_[:, 0:256])

        def nat_path(pa, p, hl, rsum, sc2):
            """Natural-layout scores -> exp(+rowsum) -> normalize -> DMA."""
            ro = (hl % 2) * 64
            for qi in range(NT):
                wf = (qi + 1) * 128 if causal else S
                ntile = (wf + 1023) // 1024
                rowt = []
                for ti in range(ntile):
                    w0 = ti * 1024
                    w1 = min(wf, w0 + 1024)
                    ps_ = pp.tile([128, 1024], F32, name="sa", tag="sa",
                                  bufs=2)
                    rowt.append((ps_, w0, w1))
                    for c0 in range(w0, w1, 512):
                        c1 = min(w1, c0 + 512)
                        has_neg = causal and (c1 == wf)
                        nc.tensor.matmul(
                            ps_[:, c0 - w0:c1 - w0],
                            qt[p][ro:ro + 64, qi * 128:qi * 128 + 128],
                            kt[p][ro:ro + 64, c0:c1],
                            start=True, stop=not padded and not has_neg)
                        if padded:
                            nc.tensor.matmul(
                                ps_[:, c0 - w0:c1 - w0],
                                onesr[0:1, 0:128], padr[0:1, c0:c1],
                                start=False, stop=not has_neg)
                        if has_neg:
                            nc.tensor.matmul(
                                ps_[:, wf - 128 - w0:wf - w0],
                                idnr[:], neg[:], start=False, stop=True)
                # exp with free row-sum accumulation
                alb = pa.tile([128, S], F32, name="alb", tag="alb", bufs=3)
                col = hl * 16 + qi
                for ti, (ps_, w0, w1) in enumerate(rowt):
                    acc = (rsum[:, col:col + 1] if ntile == 1
                           else sc2[:, ti:ti + 1])
                    nc.scalar.activation(
                        alb[:, w0:w1], ps_[:, 0:w1 - w0], AFT.Exp,
                        scale=0.125, accum_out=acc)
                if ntile > 1:
                    nc.vector.tensor_tensor(
                        out=rsum[:, col:col + 1], in0=sc2[:, 0:1],
                        in1=sc2[:, 1:2], op=mybir.AluOpType.add)
                rc = pa.tile([128, 1], F32, name="rc", tag="rc", bufs=3)
                nc.vector.reciprocal(out=rc[:], in_=rsum[:, col:col + 1])
                nc.vector.tensor_scalar_mul(alb[:, 0:wf], alb[:, 0:wf],
                                            rc[:, 0:1])
                nc.sync.dma_start(
                    out=ALP_[hl, qi * 128:qi * 128 + 128, 0:wf],
                    in_=alb[:, 0:wf])

        def bcast_recip(pa, hl, rsum):
            """[64, S] broadcast tile of 1/rowsum for one head."""
            bcast = pa.tile([64, S], F32, name="bcast", tag="bcast", bufs=2)
            pt_ = pp.tile([128, 512], F32, name="ptr", tag="mm", bufs=2)
            nc.tensor.transpose(pt_[0:16, 0:128],
                                rsum[:, hl * 16:hl * 16 + 16], idn32[:])
            rrt = pa.tile([16, 128], R32, name="rrt", tag="rrt", bufs=2)
            with nc.allow_low_precision(reason="f32r recip feeds matmul"):
                nc.vector.reciprocal(out=rrt[:], in_=pt_[0:16, 0:128])
            row = pa.tile([1, S], R32, name="row", tag="row", bufs=2)
            nc.sync.dma_start(out=row[:], in_=rrt[:])
            for sc in range(4):
                pb_ = pp.tile([128, 512], F32, name="pb", tag="mm", bufs=2)
                nc.tensor.matmul(pb_[0:64, :], onesr[0:1, 0:64],
                                 row[0:1, sc * 512:sc * 512 + 512],
                                 start=True, stop=True)
                nc.vector.tensor_copy(
                    out=bcast[:, sc * 512:sc * 512 + 512], in_=pb_[0:64, :])
            return bcast

        def t_path(pa, p, bcasts, ctxn):
            """Transposed scores -> exp -> context (per-head psum)."""
            for j in range(4):
                pctx = [pp.tile([64, 512], F32, name=f"pctx{ih}",
                                tag=f"ctx{ih}", bufs=1) for ih in range(2)]
                nki = (4 * j + 4) if causal else NT
                for ki in range(nki):
                    col0 = max(0, 128 * ki - 512 * j) if causal else 0
                    diag = causal and (128 * ki >= 512 * j)
                    st_ = pp.tile([128, 1024], F32, name="st", tag="sa",
                                  bufs=2)
                    ex_ = pa.tile([128, 1024], R32, name="ex", tag="ex",
                                  bufs=4)
                    for ih, hl in enumerate((2 * p, 2 * p + 1)):
                        base = ih * 512
                        ro = (hl % 2) * 64
                        nc.tensor.matmul(
                            st_[:, base + col0:base + 512],
                            kt[p][ro:ro + 64, ki * 128:ki * 128 + 128],
                            qt[p][ro:ro + 64, j * 512 + col0:j * 512 + 512],
                            start=True, stop=True)
                    nc.scalar.activation(
                        ex_[:, col0:1024], st_[:, col0:1024], AFT.Exp,
                        scale=0.125, bias=padb[:, ki:ki + 1])
                    if diag:
                        for base in (col0, 512 + col0):
                            nc.vector.tensor_tensor(
                                out=ex_[:, base:base + 128],
                                in0=ex_[:, base:base + 128], in1=mskt[:],
                                op=mybir.AluOpType.mult)
                    for ih, hl in enumerate((2 * p, 2 * p + 1)):
                        base = ih * 512
                        nc.tensor.matmul(
                            pctx[ih][0:64, col0:512],
                            vsb[:, ki * 256 + hl * 64:ki * 256 + hl * 64 + 64],
                            ex_[:, base + col0:base + 512],
                            start=(ki == 0), stop=(ki == nki - 1))
                for ih, hl in enumerate((2 * p, 2 * p + 1)):
                    nc.vector.tensor_tensor(
                        out=ctxn[hl][:, j * 512:j * 512 + 512],
                        in0=pctx[ih][0:64, :],
                        in1=bcasts[ih][:, j * 512:j * 512 + 512],
                        op=mybir.AluOpType.mult)

        def attn_phase(pa):
            rsum = pa.tile([128, 64], F32, name="rsum")
            sc2 = pa.tile([128, 2], F32, name="sc2", bufs=3)
            ctxn = [pa.tile([64, S], R32, name=f"ctxn{h}", tag="ctxn",
                            bufs=4) for h in range(4)]
            for p in range(2):
                for hl in (2 * p, 2 * p + 1):
                    nat_path(pa, p, hl, rsum, sc2)
            for p in range(2):
                bcasts = [bcast_recip(pa, hl, rsum)
                          for hl in (2 * p, 2 * p + 1)]
                t_path(pa, p, bcasts, ctxn)
            # output projection: partial out rows, 4-head accumulation
            for st in range(NT):
                outst = pa.tile([128, 1024], F32, name="outst", tag="outst",
                                bufs=2)
                for oc in range(2):
                    po_ = pp.tile([128, 512], F32, name="po", tag="mm",
                                  bufs=2)
                    for hl in range(4):
                        nc.tensor.matmul(
                            po_[:], ctxn[hl][:, st * 128:st * 128 + 128],
                            wo[:, hl * 1024 + oc * 512:
                               hl * 1024 + oc * 512 + 512],
                            start=(hl == 0), stop=(hl == 3))
                    nc.vector.tensor_copy(
                        out=outst[:, oc * 512:oc * 512 + 512], in_=po_[:])
                nc.sync.dma_start(out=OUT_[st * 128:st * 128 + 128, :],
                                  in_=outst[:])

        def body():
            load_consts()
            with tc.tile_pool(name="px", bufs=1) as px:
                qkv_phase(px)
            with tc.tile_pool(name="pa", bufs=1) as pa:
                attn_phase(pa)
            if timing:
                prb = pw.tile([128, 16], F32, name="prb")
                nc.vector.memset(prb[:], 1.0)
                nc.sync.dma_start(out=PRB_[:], in_=prb[:])

        if timing and nreps > 1:
            with tc.For_i(0, nreps, 1):
                body()
        else:
            body()
        est.close()
    nc.finalize()
    return nc


def _get(causal, padded, biased, timing=False, nreps=1):
    key = (causal, padded, biased, timing, nreps)
    if key not in _BUILD_CACHE:
        _BUILD_CACHE[key] = _build(causal, padded, biased, timing, nreps)
    return _BUILD_CACHE[key]


def host_prep(X, Wq, bq, Wk, bk, Wv, bv, Wo, bo, padding_mask, causal_mask):
    X = np.asarray(X, dtype=np.float32)
    padding_mask = np.asarray(padding_mask)
    causal = bool(np.asarray(causal_mask).item())
    padded = bool(np.any(padding_mask == 1))

    neg = np.triu(np.full((128, 128), NEGV, np.float32), k=1)
    mskt = np.triu(np.ones((128, 128), np.float32), k=0)
    biased = bool(np.any(np.asarray(bq)) or np.any(np.asarray(bk))
                  or np.any(np.asarray(bv)))
    idn = np.eye(128, dtype=np.float32)
    in_maps = []
    for c in range(8):
        b, g = c // 4, c % 4
        hs = slice(256 * g, 256 * (g + 1))
        padv = (padding_mask[b] == 1).astype(np.float32) * (8.0 * NEGV)
        m = {
            "XT": np.ascontiguousarray(X[b].T),
            "WQT": np.ascontiguousarray(np.asarray(Wq, dtype=np.float32)[hs, :].T),
            "WKT": np.ascontiguousarray(np.asarray(Wk, dtype=np.float32)[hs, :].T),
            "WVT": np.ascontiguousarray(np.asarray(Wv, dtype=np.float32)[hs, :].T),
            "WOT": np.ascontiguousarray(np.asarray(Wo, dtype=np.float32)[:, hs].T),
            "BIA": np.concatenate([
                np.asarray(bq, dtype=np.float32)[hs],
                np.asarray(bk, dtype=np.float32)[hs],
                np.asarray(bv, dtype=np.float32)[hs]]).reshape(1, 768),
            "PADB": np.ascontiguousarray(padv.reshape(NT, 128).T),
            "PADR": (padv / 8.0).reshape(1, S).astype(np.float32),
            "NEG": neg,
            "MSKT": mskt,
            "IDNR": idn,
            "IDN32": idn,
        }
        in_maps.append(m)
    return in_maps, causal, padded, biased


def kernel(X, Wq, bq, Wk, bk, Wv, bv, Wo, bo, padding_mask, causal_mask):
    in_maps, causal, padded, biased = host_prep(
        X, Wq, bq, Wk, bk, Wv, bv, Wo, bo, padding_mask, causal_mask)
    nc = _get(causal, padded, biased)
    res = run_bass_kernel_spmd(nc, in_maps, list(range(8)))
    out = np.zeros((B, S, D), np.float32)
    alpha = np.zeros((B, H, S, S), np.float32)
    for c in range(8):
        b, g = c // 4, c % 4
        out[b] += res.results[c]["OUT"]
        alpha[b, 4 * g:4 * g + 4] = res.results[c]["ALP"]
    out += np.asarray(bo, dtype=np.float32)[None, None, :]
    return out, alpha
